# revision 2
# baseline (speedup 1.0000x reference)
"""Trainium2 Bass kernel for nn_MultiHeadedAttention (B=2, H=16, S=2048, d=64).

Sharding: data-parallel over batch x tensor-parallel over heads.
8 cores = 2 batch groups x 4 head-groups (4 heads each).

v2 restructure over the baseline (175us):
  - Host pre-arranges every weight into its SBUF layout so all input DMAs are
    fully contiguous slabs; loads are split across BOTH HWDGE rings (sync +
    scalar) with arrival ordered by first use; wo is cast to bf16 on host.
  - PE warm-up: a burst of zero matmuls at t=0 (only dep: one memset) keeps
    the PE busy while inputs stream in, flipping the HAM clock gate to 8/8
    before real work starts; a dummy exp triggers the ACT table load early.
  - Fine-grained software pipelining: projection work is a filler queue
    consumed inside the attention phases via a per-chunk ACT-vs-PE deficit
    counter, so the PE always has independent matmuls while ScalarE runs the
    exp stream (the attention inner loop is ACT-bound: ~1147ns exp vs ~570ns
    of PE work per 512-chunk).
  - All four attention phases normalize + transpose eagerly per q-tile
    (kt == jq); O-projection tiles are emitted as soon as both head-pairs
    finish a q-tile, spreading O-proj + output DMA across the whole run.
  - Output is shipped bf16 (halves out-DMA; host sums partials in f32).
  - All PSUM->SBUF copies pinned to VectorE (never ScalarE - exp is critical).
  - PSUM budget: hs 3 banks + scores 2x2 banks + 1 shared filler bank = 8.
Attention math is identical to the baseline: fp32->bf16 projections, 2-head
row-group-packed score matmuls, one exp op per (kt, 512-chunk) covering both
heads (scale=1/8, no max subtraction), tri-mask on diagonal blocks, PV
accumulation with a ones column for the softmax denominator, eager
reciprocal+scale normalization.  Host adds the exact (b_V @ W_O + b_O) row.
"""

import math
from collections import deque
from contextlib import ExitStack

import numpy as np
import ml_dtypes

import concourse.bass as bass
import concourse.mybir as mybir
import concourse.tile as tile
from concourse import bacc, bass_utils

F32 = mybir.dt.float32
BF16 = mybir.dt.bfloat16
EXP = mybir.ActivationFunctionType.Exp

B, S, D = 2, 2048, 1024
NH, HD = 16, 64
NCORES = 8
GROUPS = NCORES // B          # 4 head-groups per batch
HPC = NH // GROUPS            # 4 heads per core
M = HPC * HD                  # 256 local head-dims per core
P = 128
KC = D // P                   # 8 contraction chunks
NT = S // P                   # 16 q/s tiles
SCALE = 1.0 / math.sqrt(HD)   # 0.125
N_WARMUP = 12                 # zero-MM PE warm-up burst


def build_kernel():
    nc = bacc.Bacc("TRN2", target_bir_lowering=False)

    xT_d = nc.dram_tensor("xT", [D, S], BF16, kind="ExternalInput")
    wq_d = nc.dram_tensor("wq", [P, KC, M], BF16, kind="ExternalInput")
    wk_d = nc.dram_tensor("wk", [P, KC, M], BF16, kind="ExternalInput")
    wv_d = nc.dram_tensor("wv", [P, KC, M], BF16, kind="ExternalInput")
    wo_d = nc.dram_tensor("wo", [P, 2, D], BF16, kind="ExternalInput")
    bq_d = nc.dram_tensor("bq", [P, 2], F32, kind="ExternalInput")
    bk_d = nc.dram_tensor("bk", [P, 2], F32, kind="ExternalInput")
    tri_d = nc.dram_tensor("tri", [P, P], BF16, kind="ExternalInput")
    ident_d = nc.dram_tensor("ident", [P, P], BF16, kind="ExternalInput")
    out_d = nc.dram_tensor("out", [S, D], BF16, kind="ExternalOutput")

    with tile.TileContext(nc) as tc, ExitStack() as ctx:
        big = ctx.enter_context(tc.tile_pool(name="big", bufs=1))
        exp_pool = ctx.enter_context(tc.tile_pool(name="expp", bufs=8))
        outcp = ctx.enter_context(tc.tile_pool(name="outcp", bufs=4))
        recip_pool = ctx.enter_context(tc.tile_pool(name="recipp", bufs=2))

        # ---- persistent SBUF tiles ----
        xT_sb = big.tile([P, KC, S], BF16)
        wq_sb = big.tile([P, KC, M], BF16)
        wk_sb = big.tile([P, KC, M], BF16)
        wv_sb = big.tile([P, KC, M], BF16)
        wo_sb = big.tile([P, 2, D], BF16)
        bq_sb = big.tile([P, 2], F32)
        bk_sb = big.tile([P, 2], F32)
        qT_sb = big.tile([P, 2, S], BF16)
        kT_sb = big.tile([P, 2, S], BF16)
        v_sb = big.tile([P, NT, HPC, HD + 1], BF16)
        hs_sb = big.tile([P, NT, M], BF16)
        hsT_sb = big.tile([P, 2, NT, P], BF16)
        tri_sb = big.tile([P, P], BF16)
        ident_sb = big.tile([P, P], BF16)
        zz_sb = big.tile([1, 512], BF16)
        dummy_sb = big.tile([1, 2], BF16)

        nc.vector.memset(zz_sb[:], 0.0)
        nc.vector.memset(v_sb[:, :, :, HD : HD + 1], 1.0)
        # trigger the ACT exp-table load (~2.7us) before the DMA/proj prefix
        nc.scalar.activation(dummy_sb[:], zz_sb[0:1, 0:2], EXP, scale=SCALE)

        # ---- input DMAs: contiguous slabs, split across both HWDGE rings,
        # ordered by first use ----
        nc.sync.dma_start(wq_sb[:], wq_d.ap())
        nc.scalar.dma_start(wk_sb[:], wk_d.ap())
        nc.scalar.dma_start(bk_sb[:], bk_d.ap())
        nc.scalar.dma_start(bq_sb[:], bq_d.ap())
        for kc in range(KC):
            eng = nc.sync if kc % 2 == 0 else nc.scalar
            eng.dma_start(xT_sb[:, kc, :], xT_d.ap()[P * kc : P * (kc + 1), :])
        nc.scalar.dma_start(tri_sb[:], tri_d.ap())
        nc.scalar.dma_start(wv_sb[:], wv_d.ap())
        nc.scalar.dma_start(ident_sb[:], ident_d.ap())
        nc.scalar.dma_start(wo_sb[:], wo_d.ap())

        # ---- PSUM pools ----
        attn_ps = ctx.enter_context(tc.tile_pool(name="attn_ps", bufs=1, space="PSUM"))
        fil_ps = ctx.enter_context(tc.tile_pool(name="fil_ps", bufs=1, space="PSUM"))

        # ---- PE warm-up burst (deps: only the zz memset) ----
        for i in range(N_WARMUP):
            wps = fil_ps.tile([P, 512], F32, tag="fil", bufs=1, name=f"wu{i}")
            nc.tensor.matmul(
                wps[:], lhsT=zz_sb[0:1, 0:P], rhs=zz_sb[0:1, 0:512],
                start=True, stop=True,
            )

        # ---- filler primitives (each allocates one fil-bank tile) ----
        def proj_qk_tile(which, hp, nq):
            w_sb, t_sb, b_sb = (
                (wq_sb, qT_sb, bq_sb) if which == "q" else (wk_sb, kT_sb, bk_sb)
            )
            ps = fil_ps.tile([P, 512], F32, tag="fil", bufs=1, name=f"p{which}{hp}{nq}")
            for kc in range(KC):
                nc.tensor.matmul(
                    ps[:],
                    lhsT=w_sb[:, kc, P * hp : P * (hp + 1)],
                    rhs=xT_sb[:, kc, 512 * nq : 512 * (nq + 1)],
                    start=(kc == 0),
                    stop=(kc == KC - 1),
                )
            nc.vector.tensor_scalar_add(
                t_sb[:, hp, 512 * nq : 512 * (nq + 1)], ps[:], b_sb[:, hp : hp + 1]
            )

        def proj_v_tile(st):
            ps = fil_ps.tile([P, M], F32, tag="fil", bufs=1, name=f"pv{st}")
            for kc in range(KC):
                nc.tensor.matmul(
                    ps[:],
                    lhsT=xT_sb[:, kc, P * st : P * (st + 1)],
                    rhs=wv_sb[:, kc, :],
                    start=(kc == 0),
                    stop=(kc == KC - 1),
                )
            nc.vector.tensor_copy(
                v_sb[:, st, :, 0:HD], ps[:].rearrange("p (h d) -> p h d", h=HPC)
            )

        def oproj_tile(st, dc):
            ps = fil_ps.tile([P, 512], F32, tag="fil", bufs=1, name=f"o{st}{dc}")
            for hp in range(2):
                nc.tensor.matmul(
                    ps[:],
                    lhsT=hsT_sb[:, hp, st, :],
                    rhs=wo_sb[:, hp, 512 * dc : 512 * (dc + 1)],
                    start=(hp == 0),
                    stop=(hp == 1),
                )
            o_sb = outcp.tile([P, 512], BF16, tag="o", name=f"oc{st}{dc}")
            nc.vector.tensor_copy(o_sb[:], ps[:])
            nc.sync.dma_start(
                out_d.ap()[P * st : P * (st + 1), 512 * dc : 512 * (dc + 1)], o_sb[:]
            )

        def transp_tile(hp, jq):
            tp = fil_ps.tile([P, P], BF16, tag="fil", bufs=1, name=f"tp{hp}{jq}")
            nc.tensor.transpose(tp[:], hs_sb[:, jq, P * hp : P * (hp + 1)], ident_sb[:])
            nc.vector.tensor_copy(hsT_sb[:, hp, jq, :], tp[:])

        # ---- filler queue: (pe_cost_ns, closure), consumed deficit-driven ----
        filler = deque()

        def fill(budget):
            while filler and budget > 0:
                cost, fn = filler.popleft()
                fn()
                budget -= cost
            return budget

        QK_COST = 8 * 219   # 8 N=512 matmuls
        V_COST = 8 * 113    # 8 N=256 matmuls
        OP_COST = 2 * 219 + 60

        # ---- attention phase ----
        def attn_phase(hp, ph):
            qlo, qhi = 1024 * ph, 1024 * (ph + 1)
            hs_tiles = [
                attn_ps.tile([P, 455], F32, tag="hs", bufs=3, name=f"hs{hp}{ph}{i}")
                for i in range(3)
            ]

            def slot(eta, jql):
                if jql < 7:
                    return hs_tiles[eta], 65 * jql
                return hs_tiles[2], 65 * eta

            for t in hs_tiles:
                nc.tensor.matmul(
                    t[:, 0:455],
                    lhsT=zz_sb[0:1, 0:P],
                    rhs=zz_sb[0:1, 0:455],
                    start=True,
                    stop=True,
                    skip_group_check=True,
                )
            deficit = 0.0
            for kt in range(qhi // P):
                qstart = max(qlo, P * kt)
                for q0 in range(qstart, qhi, 512):
                    w = min(512, qhi - q0)
                    s_ps = attn_ps.tile(
                        [P, 1024], F32, tag="sc", bufs=2, name=f"sc{hp}{ph}{kt}{q0}"
                    )
                    for eta in range(2):
                        prow = slice(HD * eta, HD * (eta + 1))
                        nc.tensor.matmul(
                            s_ps[:, 512 * eta : 512 * eta + w],
                            lhsT=kT_sb[prow, hp, P * kt : P * (kt + 1)],
                            rhs=qT_sb[prow, hp, q0 : q0 + w],
                            start=True,
                            stop=True,
                        )
                    e_sb = exp_pool.tile([P, 1024], BF16, tag="e", name=f"e{kt}{q0}")
                    pair = s_ps[:].rearrange("p (g f) -> p g f", g=2)[:, :, 0:w]
                    epair = e_sb[:].rearrange("p (g f) -> p g f", g=2)[:, :, 0:w]
                    nc.scalar.activation(epair, pair, EXP, scale=SCALE)
                    if q0 == P * kt:  # chunk starts at the diagonal block
                        nc.vector.tensor_tensor(
                            e_sb[:].rearrange("p (g f) -> p g f", g=2)[:, :, 0:P],
                            e_sb[:].rearrange("p (g f) -> p g f", g=2)[:, :, 0:P],
                            tri_sb[:]
                            .rearrange("p (o f) -> p o f", o=1)
                            .broadcast_to([P, 2, P]),
                            op=mybir.AluOpType.mult,
                        )
                    nblk = 0
                    for eta in range(2):
                        h = 2 * hp + eta
                        for jq in range(q0 // P, (q0 + w) // P):
                            t, col = slot(eta, jq - 8 * ph)
                            nc.tensor.matmul(
                                t[:, col : col + HD + 1],
                                lhsT=e_sb[
                                    :,
                                    512 * eta + P * jq - q0 : 512 * eta + P * jq - q0 + P,
                                ],
                                rhs=v_sb[:, kt, h, :],
                                start=False,
                                stop=(kt == jq),
                                skip_group_check=True,
                            )
                            nblk += 1
                    deficit += (2 * w + 352) / 1.2 - (w / 2.4 + nblk * 45)
                    deficit = fill(deficit)
                if kt >= 8 * ph:
                    # slot jq=kt complete: normalize, transpose, maybe O-proj
                    jql = kt - 8 * ph
                    recip_t = recip_pool.tile(
                        [P, 2], F32, tag="re", bufs=8, name=f"re{hp}{ph}{kt}"
                    )
                    for eta in range(2):
                        h = 2 * hp + eta
                        t, col = slot(eta, jql)
                        nc.vector.reciprocal(
                            recip_t[:, eta : eta + 1], t[:, col + HD : col + HD + 1]
                        )
                        nc.vector.tensor_scalar_mul(
                            hs_sb[:, kt, HD * h : HD * (h + 1)],
                            t[:, col : col + HD],
                            recip_t[:, eta : eta + 1],
                        )
                    transp_tile(hp, kt)
                    deficit -= 275
                    if hp == 1:
                        oproj_tile(kt, 0)
                        oproj_tile(kt, 1)
                        deficit -= 2 * OP_COST

        # ---- prefix: minimum to start attn(0,0), everything else is filler ----
        proj_qk_tile("k", 0, 0)
        proj_qk_tile("q", 0, 0)
        proj_qk_tile("q", 0, 1)
        for st in range(4):
            proj_v_tile(st)

        for item in (
            [("k", 0, 1)]
            + [("v", st) for st in range(4, 8)]
            + [("q", 0, 2), ("q", 0, 3)]
            + [("v", st) for st in range(8, 12)]
            + [("k", 0, 2)]
            + [("v", st) for st in range(12, 16)]
            + [("k", 0, 3)]
            + [("k", 1, 0), ("q", 1, 0), ("q", 1, 1), ("k", 1, 1)]
            + [("q", 1, 2), ("q", 1, 3), ("k", 1, 2), ("k", 1, 3)]
        ):
            if item[0] == "v":
                st = item[1]
                filler.append((V_COST, lambda st=st: proj_v_tile(st)))
            else:
                which, hp, nq = item
                filler.append(
                    (QK_COST, lambda w=which, h=hp, n=nq: proj_qk_tile(w, h, n))
                )

        attn_phase(0, 0)
        attn_phase(0, 1)
        attn_phase(1, 0)
        attn_phase(1, 1)
        fill(float("inf"))  # drain any leftover filler

    nc.compile()
    return nc


_NC = None


def _get_nc():
    global _NC
    if _NC is None:
        _NC = build_kernel()
    return _NC


def _tri_upper(n=P):
    m = np.zeros((n, n), np.float32)
    iu = np.triu_indices(n, 0)
    m[iu] = 1.0
    return m.astype(ml_dtypes.bfloat16)


def kernel(x, W_Q, W_K, W_V, W_O, b_Q, b_K, b_V, b_O, _trace=False):
    x = np.asarray(x, np.float32)
    W_Q, W_K = np.asarray(W_Q, np.float32), np.asarray(W_K, np.float32)
    W_V, W_O = np.asarray(W_V, np.float32), np.asarray(W_O, np.float32)
    b_Q, b_K = np.asarray(b_Q, np.float32), np.asarray(b_K, np.float32)
    b_V, b_O = np.asarray(b_V, np.float32), np.asarray(b_O, np.float32)

    nc = _get_nc()
    tri = _tri_upper()
    ident = np.eye(P, dtype=np.float32).astype(ml_dtypes.bfloat16)
    xT_b = [np.ascontiguousarray(x[b].T).astype(ml_dtypes.bfloat16) for b in range(B)]

    def warr(W, cols):  # [D, Mloc] -> [P, KC, Mloc] contiguous
        return np.ascontiguousarray(
            W[:, cols].reshape(KC, P, M).transpose(1, 0, 2)
        ).astype(ml_dtypes.bfloat16)

    in_maps = []
    for core in range(NCORES):
        b, g = core // GROUPS, core % GROUPS
        cols = slice(M * g, M * (g + 1))
        in_maps.append(
            {
                "xT": xT_b[b],
                "wq": warr(W_Q, cols),
                "wk": warr(W_K, cols),
                "wv": warr(W_V, cols),
                "wo": np.ascontiguousarray(
                    W_O[cols, :].reshape(2, P, D).transpose(1, 0, 2)
                ).astype(ml_dtypes.bfloat16),
                "bq": np.ascontiguousarray(b_Q[cols].reshape(2, P).T),
                "bk": np.ascontiguousarray(b_K[cols].reshape(2, P).T),
                "tri": tri,
                "ident": ident,
            }
        )
    res = bass_utils.run_bass_kernel_spmd(
        nc, in_maps, core_ids=list(range(NCORES)), trace=_trace
    )
    const_row = (b_V @ W_O + b_O).astype(np.float32)  # exact: sum(softmax)=1
    out = np.zeros((B, S, D), np.float32)
    for b in range(B):
        acc = res.results[b * GROUPS]["out"].astype(np.float32)
        for g in range(1, GROUPS):
            acc = acc + res.results[b * GROUPS + g]["out"].astype(np.float32)
        out[b] = acc + const_row
    if _trace:
        kernel.last_results = res
    return out
